# revision 10
# baseline (speedup 1.0000x reference)
"""Trainium2 Bass kernel for nn_CCNLoss (v8: custom-DVE fused abs-diff-sum).

loss = mean(|p - t|) + 0.5 * sum(arccos(clip(cos, -1+1e-7, 1-1e-7))) + |crm(p) - crm(t)|

where cos[h,w] = sum_c sab_c / sqrt(saa_c * sbb_c), s** = sum_b of pt/pp/tt.

Algebraic facts (validated numerically against the reference):
  * crm(img) = mean(softmax(X, 0)) == 1/m exactly -> the crm term is 0; dropped.
  * arccos(x) = 2*atan(sqrt((1-x)/(1+x))); the 2 cancels the 0.5 weight.
  * u' = min(cos, CLIP) in f32; t1 = 1-u' (exact by Sterbenz; == 1-CLIP when
    clipped), t2 = 1+u'; theta = 2*atan(t1 * rsqrt(t1*t2)).
  * fp16 inputs/products perturb the loss ~3e-5 relative (measured).
  * sum|p-t| runs as one fused custom-DVE op per channel:
    out = max(p-t, t-p), accum_out += out  (registered additively into the
    concourse custom-DVE table; row space [1, 0x20) is documented free).

Structure per core (h-slab of 128 rows on 128 partitions):
  * HBM [C, HC, NCH, B, WC] fp16; channel DMAs up-front (c2 split per chunk).
  * PE: b-sums via identity-weight accumulating matmuls + warmup filler.
  * ACT: squares c0/c2k0, rsqrt pairs, chunk rsqrts, one table preload, one
    swap to the trig set for the final arctan.
  * V: pt muls, c1/c2k1 squares, fused |d| ops, cos muls, chain TS ops.
  * Pool: inv muls, chunk-0 adds/mm.
"""

import numpy as np
from contextlib import ExitStack
from operator import add as _op_add

import concourse.bass as bass
import concourse.bacc as bacc
import concourse.tile as tile
from concourse import mybir
from concourse import dve_ops as _dvo
import concourse.bass_utils as _bu
from concourse.bass_utils import run_bass_kernel_spmd
from concourse.dve_spec import Spec as _Spec, Src0 as _S0, Src1 as _S1, \
    maxx as _maxx, Zero as _Zero, lower as _lower
from concourse.dve_uop import DveOpSpec as _DveOpSpec

B, C, H, W = 4, 3, 1024, 1024
NCORES = 8
HC = H // NCORES
P = 128
WC = 512
NCH = 2

F32 = mybir.dt.float32
F16 = mybir.dt.float16
AF = mybir.ActivationFunctionType
OP = mybir.AluOpType
AX = mybir.AxisListType

CLIP_HI = float(np.float32(1.0 - 1e-7))
N_WARM = 10

# ---- custom DVE op: out = |in0 - in1|, accum_out = sum(out) --------------


def _absdiff_ref(in0, in1, c0, c1, c2):
    b = np.abs(in0.astype(np.float32) - in1.astype(np.float32)).astype(
        np.float32
    )
    return b, b.reshape(b.shape[0], -1).sum(axis=-1, keepdims=True)


def _register_absdiff():
    name = "ABS_DIFF_REDUCE_ANT"
    if name in _dvo._SUB_OPCODE_FOR_NAME:
        return next(o for o in _dvo.OPS if o.name == name)
    spec = _Spec(
        body=_maxx(_S0 - _S1, _S1 - _S0),
        accum=_op_add,
        accum_init=_Zero,
        reference=_absdiff_ref,
    )
    row = _dvo._CUSTOM_DVE_ROW_BASE + len(_dvo.OPS)
    shas = {}
    for ver in ("v3", "v4"):
        s = _DveOpSpec(name=name, opcode=row, uops=_lower(spec, ver=ver),
                       rd1_en=True)
        shas[ver] = s.sha(ver)
    op = _dvo.DveOp(name, spec, subdim=False, uops_sha=shas,
                    perf_en={"v3": True, "v4": True})
    _dvo._SUB_OPCODE_FOR_NAME[name] = row
    _dvo.OPS = (*_dvo.OPS, op)
    _bu.OPS = _dvo.OPS
    _dvo.CUSTOM_DVE_SPECS[name] = spec
    return op


ABSDIFF = _register_absdiff()

_CACHE = {}


def _body(tc, pred, targ, identf16, res_out):
    nc = tc.nc
    with ExitStack() as ctx:
        inpool = ctx.enter_context(tc.tile_pool(name="inp", bufs=1))
        prodp = ctx.enter_context(tc.tile_pool(name="prod", bufs=2))
        scrp = ctx.enter_context(tc.tile_pool(name="scr", bufs=1))
        work = ctx.enter_context(tc.tile_pool(name="work", bufs=2))
        consts = ctx.enter_context(tc.tile_pool(name="consts", bufs=1))
        psum = ctx.enter_context(tc.tile_pool(name="ps", bufs=2, space="PSUM"))
        outp = ctx.enter_context(tc.tile_pool(name="outp", bufs=1))

        idw = consts.tile([P, P], F16)
        nc.sync.dma_start(out=idw, in_=identf16)

        # res: col c (c<2) / 2+k (c2 per chunk) = sum|d|; col 7 = atan sum
        res = outp.tile([P, 8], F32)
        nc.gpsimd.memset(res, 0.0)

        ptc = [inpool.tile([P, 2, NCH, B, WC], F16, name=f"ptc{c}", bufs=1)
               for c in range(C)]
        for c in range(C):
            if c < C - 1:
                nc.sync.dma_start(out=ptc[c][:, 0], in_=pred[c])
                nc.sync.dma_start(out=ptc[c][:, 1], in_=targ[c])
            else:
                for k in range(NCH):
                    nc.sync.dma_start(out=ptc[c][:, 0, k], in_=pred[c, :, k])
                    nc.sync.dma_start(out=ptc[c][:, 1, k], in_=targ[c, :, k])

        # force the Abs_reciprocal_sqrt table set before any Square lands
        wsrc = consts.tile([P, WC], F16)
        nc.gpsimd.memset(wsrc, 0.0)
        tdum = consts.tile([P, 1], F32)
        nc.scalar.activation(tdum, wsrc[:, 0:1], AF.Abs_reciprocal_sqrt)

        warm = psum.tile([P, WC], F32, tag="warm", bufs=1)
        for _ in range(N_WARM):
            nc.tensor.matmul(warm, idw, wsrc, start=True, stop=True)

        scr = scrp.tile([P, 2, NCH, B, WC], F16, name="scr")
        prod = {}

        def products(c, k=None):
            """pt mul + squares for channel c (k=None: both chunks)."""
            if c not in prod:
                prod[c] = prodp.tile([P, 3, NCH, B, WC], F16, tag="prod",
                                     name=f"pr{c}", bufs=2)
            pr = prod[c]
            ks = slice(None) if k is None else slice(k, k + 1)
            pk = ptc[c][:, 0, ks]
            tk = ptc[c][:, 1, ks]
            nc.vector.tensor_mul(pr[:, 0, ks], pk, tk)
            sq_eng = 'v' if (c == 1 or (c == 2 and k == 1)) else 's'
            if sq_eng == 's':
                nc.scalar.square(pr[:, 1:3, ks], ptc[c][:, :, ks])
            else:
                nc.vector.tensor_mul(pr[:, 1:3, ks], ptc[c][:, :, ks],
                                     ptc[c][:, :, ks])

        def absred(c, k=None):
            """fused |p-t| + accumulate on V via the custom DVE op."""
            ks = slice(None) if k is None else slice(k, k + 1)
            col = c if c < 2 else 2 + (k or 0)
            nc.vector._custom_dve(
                ABSDIFF,
                out=scr[:, 0, ks].opt(),
                in0=ptc[c][:, 0, ks].opt(),
                in1=ptc[c][:, 1, ks].opt(),
                accum_out=res[:, col:col + 1],
            )

        cosq = {k: work.tile([P, C, WC], F16, tag=f"cosq{k}", bufs=1,
                             name=f"cosq{k}")
                for k in range(NCH)}

        def pe_unit(c, k, ws=None):
            w0, w1 = ws if ws else (0, WC)
            ps = psum.tile([P, 3, WC], F32, tag="ps", name=f"ps{c}{k}")
            for q in range(3):
                for b in range(B):
                    nc.tensor.matmul(
                        ps[:, q, w0:w1], idw, prod[c][:, q, k, b, w0:w1],
                        start=(b == 0), stop=(b == B - 1),
                    )
            return ps

        def tail(c, k, ps, ws=None):
            w0, w1 = ws if ws else (0, WC)
            wsl = slice(w0, w1)
            rinv = work.tile([P, 2, WC], F16, tag="rinv", name=f"ri{c}{k}{w0}")
            nc.scalar.activation(rinv[:, :, wsl], ps[:, 1:3, wsl],
                                 AF.Abs_reciprocal_sqrt)
            inv = work.tile([P, WC], F16, tag="inv", name=f"iv{c}{k}{w0}")
            nc.gpsimd.tensor_mul(inv[:, wsl], rinv[:, 0, wsl], rinv[:, 1, wsl])
            nc.vector.tensor_mul(cosq[k][:, c, wsl], ps[:, 0, wsl],
                                 inv[:, wsl])

        chn = {}
        ssb = outp.tile([P, NCH, WC], F32)

        def chain(k, ws=None, eng='v'):
            w0, w1 = ws if ws else (0, WC)
            s = slice(w0, w1)
            t = chn.setdefault(k, dict(
                cs=work.tile([P, WC], F16, tag="cs", bufs=1, name=f"cs{k}"),
                co=work.tile([P, WC], F16, tag="co", bufs=1, name=f"co{k}"),
                u1=work.tile([P, WC], F32, tag="u1", bufs=1, name=f"u1{k}"),
                t1=work.tile([P, WC], F32, tag="t1", bufs=1, name=f"t1{k}"),
                t2=work.tile([P, WC], F32, tag="t2", bufs=1, name=f"t2{k}"),
                mm=work.tile([P, WC], F32, tag="mm", bufs=1, name=f"mm{k}"),
                sr=work.tile([P, WC], F32, tag="sr", bufs=1, name=f"sr{k}"),
            ))
            cq = cosq[k]
            add_eng = nc.gpsimd if eng == 'g' else nc.vector
            add_eng.tensor_add(t["cs"][:, s], cq[:, 0, s], cq[:, 1, s])
            add_eng.tensor_add(t["co"][:, s], t["cs"][:, s], cq[:, 2, s])
            nc.vector.tensor_scalar(
                out=t["u1"][:, s], in0=t["co"][:, s], scalar1=CLIP_HI,
                scalar2=None, op0=OP.min,
            )
            nc.vector.tensor_scalar(
                out=t["t1"][:, s], in0=t["u1"][:, s], scalar1=-1.0,
                scalar2=1.0, op0=OP.mult, op1=OP.add,
            )
            nc.vector.tensor_scalar(
                out=t["t2"][:, s], in0=t["u1"][:, s], scalar1=1.0,
                scalar2=None, op0=OP.add,
            )
            mm_eng = nc.gpsimd if eng == 'g' else nc.vector
            mm_eng.tensor_mul(t["mm"][:, s], t["t1"][:, s], t["t2"][:, s])
            nc.scalar.activation(t["sr"][:, s], t["mm"][:, s],
                                 AF.Abs_reciprocal_sqrt)

        def chain_ss(k, ws=None):
            w0, w1 = ws if ws else (0, WC)
            s = slice(w0, w1)
            nc.vector.tensor_mul(ssb[:, k, s], chn[k]["t1"][:, s],
                                 chn[k]["sr"][:, s])

        # ---------------- emission schedule ----------------
        HW_ = WC // 2
        products(0)
        products(1)
        ps00 = pe_unit(0, 0)
        tail(0, 0, ps00)
        absred(0)
        ps01 = pe_unit(0, 1)
        tail(0, 1, ps01)
        products(2, k=0)
        ps10 = pe_unit(1, 0)
        tail(1, 0, ps10)
        absred(1)
        products(2, k=1)
        ps11 = pe_unit(1, 1)
        tail(1, 1, ps11)
        absred(2, k=0)
        ps20 = pe_unit(2, 0)
        tail(2, 0, ps20)
        chain(0, eng='g')
        ps21a = pe_unit(2, 1, (0, HW_))
        tail(2, 1, ps21a, (0, HW_))
        chain(1, (0, HW_), eng='v')
        ps21b = pe_unit(2, 1, (HW_, WC))
        tail(2, 1, ps21b, (HW_, WC))
        chain_ss(0)
        absred(2, k=1)
        chain(1, (HW_, WC), eng='v')
        chain_ss(1, (0, HW_))
        chain_ss(1, (HW_, WC))

        at = outp.tile([P, NCH, WC], F16)
        nc.scalar.activation(
            out=at, in_=ssb, func=AF.Arctan, accum_out=res[:, 7:8]
        )

        nc.sync.dma_start(out=res_out, in_=res)


def _build():
    nc = bacc.Bacc(
        "TRN2", target_bir_lowering=False, debug=False, num_devices=NCORES
    )
    pred = nc.dram_tensor(
        "predictions", [C, HC, NCH, B, WC], F16, kind="ExternalInput"
    ).ap()
    targ = nc.dram_tensor(
        "targets", [C, HC, NCH, B, WC], F16, kind="ExternalInput"
    ).ap()
    identf16 = nc.dram_tensor("identf16", [P, P], F16, kind="ExternalInput").ap()
    res_out = nc.dram_tensor("partials", [P, 8], F32, kind="ExternalOutput").ap()
    with tile.TileContext(nc) as tc:
        _body(tc, pred, targ, identf16, res_out)
    nc.compile()
    return nc


def _get_nc():
    if "nc" not in _CACHE:
        _CACHE["nc"] = _build()
    return _CACHE["nc"]


def _make_in_maps(predictions, targets):
    p = np.asarray(predictions)
    t = np.asarray(targets)
    ident = np.eye(P, dtype=np.float16)
    in_maps = []
    for i in range(NCORES):
        h0 = i * HC
        ps = np.ascontiguousarray(
            p[:, :, h0 : h0 + HC, :]
            .reshape(B, C, HC, NCH, WC)
            .transpose(1, 2, 3, 0, 4)
            .astype(np.float16)
        )
        ts = np.ascontiguousarray(
            t[:, :, h0 : h0 + HC, :]
            .reshape(B, C, HC, NCH, WC)
            .transpose(1, 2, 3, 0, 4)
            .astype(np.float16)
        )
        in_maps.append({"predictions": ps, "targets": ts, "identf16": ident})
    return in_maps


def _combine(results):
    rsum = 0.0
    atsum = 0.0
    for r in results:
        part = np.asarray(r["partials"], dtype=np.float64)
        rsum += part[:, 0:4].sum()
        atsum += part[:, 7].sum()
    loss = rsum / float(B * C * H * W) + atsum
    return np.asarray(np.float32(loss))


def kernel(predictions, targets, _trace=False):
    nc = _get_nc()
    in_maps = _make_in_maps(predictions, targets)
    if _trace:
        out = run_bass_kernel_spmd(
            nc, in_maps, core_ids=list(range(NCORES)), trace=True
        )
        return _combine(out.results), out
    out = run_bass_kernel_spmd(nc, in_maps, core_ids=list(range(NCORES)))
    return _combine(out.results)


# revision 11
# speedup vs baseline: 1.2628x; 1.2628x over previous
"""Trainium2 Bass kernel for nn_CCNLoss (v9: unit-granular pipeline).

loss = mean(|p - t|) + 0.5 * sum(arccos(clip(cos, -1+1e-7, 1-1e-7))) + |crm(p) - crm(t)|

where cos[h,w] = sum_c sab_c / sqrt(saa_c * sbb_c), s** = sum_b of pt/pp/tt.

Algebraic facts (validated numerically against the reference):
  * crm(img) = mean(softmax(X, 0)) == 1/m exactly -> the crm term is 0; dropped.
  * arccos(x) = 2*atan(sqrt((1-x)/(1+x))); the 2 cancels the 0.5 weight.
  * u' = min(cos, CLIP) in f32; t1 = 1-u' (exact by Sterbenz; == 1-CLIP when
    clipped), t2 = 1+u'; theta = 2*atan(t1 * rsqrt(t1*t2)).
  * fp16 inputs/products perturb the loss ~3e-5 relative (measured).

Per-core structure (h-slab of 128 rows on 128 partitions), tuned from
measured HW rates (DVE fp16 TT 0.56ns/elem 2x, DVE f32 TS 0.84ns 2x_2p,
ACT 0.88ns, Pool ~2.3ns, PE 0.42-0.83ns/col by pstate):
  * p and t are host-packed per (channel, chunk) unit: ONE 1MB DMA per
    unit (8KB contiguous per partition-row) -> few DGE configs, first
    unit lands ~9us, all data by ~22us.
  * Per unit: pt-mul (V), fused p/t squares (V or ACT per table), d (V),
    |d|+accum (ACT Abs with accum_out), 12 PE matmuls, rsqrt pair (ACT),
    inv (Pool), cos-mul (V).
  * Last unit runs in w-halves to shorten the serial tail cascade; the
    arctan needs one table swap (Abs/Square/Rsqrt live in one set,
    preloaded via a dummy op during the DMA fill).
"""

import numpy as np
from contextlib import ExitStack

import concourse.bass as bass
import concourse.bacc as bacc
import concourse.tile as tile
from concourse import mybir
from concourse.bass_utils import run_bass_kernel_spmd

B, C, H, W = 4, 3, 1024, 1024
NCORES = 8
HC = H // NCORES
P = 128
WC = 512
NCH = 2
NU = C * NCH              # 6 (channel, chunk) units

F32 = mybir.dt.float32
F16 = mybir.dt.float16
AF = mybir.ActivationFunctionType
OP = mybir.AluOpType
AX = mybir.AxisListType

CLIP_HI = float(np.float32(1.0 - 1e-7))
N_WARM = 14

UNITS = [(0, 0), (0, 1), (1, 0), (1, 1), (2, 0), (2, 1)]
SQ_ENG = ['s', 's', 'v', 'v', 's', 'v']   # per-unit square engine
_CACHE = {}


def _body(tc, inputs, identf16, res_out):
    nc = tc.nc
    with ExitStack() as ctx:
        inpool = ctx.enter_context(tc.tile_pool(name="inp", bufs=1))
        prodp = ctx.enter_context(tc.tile_pool(name="prod", bufs=3))
        dpool = ctx.enter_context(tc.tile_pool(name="dsc", bufs=2))
        scrp = ctx.enter_context(tc.tile_pool(name="scr", bufs=1))
        work = ctx.enter_context(tc.tile_pool(name="work", bufs=2))
        consts = ctx.enter_context(tc.tile_pool(name="consts", bufs=1))
        psum = ctx.enter_context(tc.tile_pool(name="ps", bufs=2, space="PSUM"))
        outp = ctx.enter_context(tc.tile_pool(name="outp", bufs=1))

        idw = consts.tile([P, P], F16)
        nc.sync.dma_start(out=idw, in_=identf16)

        # res: col u = sum|d| of unit u; col 7 = atan sum
        res = outp.tile([P, 8], F32)

        ptk = [inpool.tile([P, 2, B, WC], F16, name=f"ptk{u}", bufs=1)
               for u in range(NU)]
        for u, (c, k) in enumerate(UNITS):
            nc.sync.dma_start(out=ptk[u], in_=inputs[c, k])

        # force the Abs_reciprocal_sqrt table set before any Square lands
        wsrc = consts.tile([P, WC], F16)
        nc.gpsimd.memset(wsrc, 0.0)
        tdum = consts.tile([P, 1], F32)
        nc.scalar.activation(tdum, wsrc[:, 0:1], AF.Abs_reciprocal_sqrt)

        warm = psum.tile([P, WC], F32, tag="warm", bufs=1)
        for _ in range(N_WARM):
            nc.tensor.matmul(warm, idw, wsrc, start=True, stop=True)

        scr = scrp.tile([P, B, WC], F16, name="scr")
        prod = {}
        dsc = {}

        def products(u, ws=None):
            """pt mul + fused squares + d for unit u over w-slice."""
            if u not in prod:
                prod[u] = prodp.tile([P, 3, B, WC], F16, tag="prod",
                                     name=f"pr{u}", bufs=3)
                dsc[u] = dpool.tile([P, B, WC], F16, tag="dsc",
                                    name=f"d{u}", bufs=2)
            w0, w1 = ws if ws else (0, WC)
            s = slice(w0, w1)
            pr = prod[u]
            pk = ptk[u][:, 0, :, s]
            tk = ptk[u][:, 1, :, s]
            nc.vector.tensor_mul(pr[:, 0, :, s], pk, tk)
            if SQ_ENG[u] == 's':
                nc.scalar.square(pr[:, 1:3, :, s], ptk[u][:, :, :, s])
            else:
                nc.vector.tensor_mul(pr[:, 1:3, :, s], ptk[u][:, :, :, s],
                                     ptk[u][:, :, :, s])
            nc.vector.tensor_sub(dsc[u][:, :, s], pk, tk)

        def absred(u):
            nc.scalar.activation(
                scr, dsc[u], AF.Abs, accum_out=res[:, u:u + 1]
            )

        cosq = {k: work.tile([P, C, WC], F16, tag=f"cosq{k}", bufs=1,
                             name=f"cosq{k}")
                for k in range(NCH)}

        def pe_unit(u, ws=None):
            w0, w1 = ws if ws else (0, WC)
            ps = psum.tile([P, 3, WC], F32, tag="ps", name=f"ps{u}{w0}")
            for q in range(3):
                for b in range(B):
                    nc.tensor.matmul(
                        ps[:, q, w0:w1], idw, prod[u][:, q, b, w0:w1],
                        start=(b == 0), stop=(b == B - 1),
                    )
            return ps

        def tail(u, ps, ws=None):
            c, k = UNITS[u]
            w0, w1 = ws if ws else (0, WC)
            wsl = slice(w0, w1)
            rinv = work.tile([P, 2, WC], F16, tag="rinv", name=f"ri{u}{w0}")
            nc.scalar.activation(rinv[:, :, wsl], ps[:, 1:3, wsl],
                                 AF.Abs_reciprocal_sqrt)
            inv = work.tile([P, WC], F16, tag="inv", name=f"iv{u}{w0}")
            nc.gpsimd.tensor_mul(inv[:, wsl], rinv[:, 0, wsl], rinv[:, 1, wsl])
            nc.vector.tensor_mul(cosq[k][:, c, wsl], ps[:, 0, wsl],
                                 inv[:, wsl])

        chn = {}
        ssb = outp.tile([P, NCH, WC], F32)

        def chain(k, ws=None, eng='v'):
            w0, w1 = ws if ws else (0, WC)
            s = slice(w0, w1)
            t = chn.setdefault(k, dict(
                cs=work.tile([P, WC], F16, tag="cs", bufs=1, name=f"cs{k}"),
                co=work.tile([P, WC], F16, tag="co", bufs=1, name=f"co{k}"),
                u1=work.tile([P, WC], F32, tag="u1", bufs=1, name=f"u1{k}"),
                t1=work.tile([P, WC], F32, tag="t1", bufs=1, name=f"t1{k}"),
                t2=work.tile([P, WC], F32, tag="t2", bufs=1, name=f"t2{k}"),
                mm=work.tile([P, WC], F32, tag="mm", bufs=1, name=f"mm{k}"),
                sr=work.tile([P, WC], F32, tag="sr", bufs=1, name=f"sr{k}"),
            ))
            cq = cosq[k]
            add_eng = nc.gpsimd if eng == 'g' else nc.vector
            add_eng.tensor_add(t["cs"][:, s], cq[:, 0, s], cq[:, 1, s])
            add_eng.tensor_add(t["co"][:, s], t["cs"][:, s], cq[:, 2, s])
            nc.vector.tensor_scalar(
                out=t["u1"][:, s], in0=t["co"][:, s], scalar1=CLIP_HI,
                scalar2=None, op0=OP.min,
            )
            nc.vector.tensor_scalar(
                out=t["t1"][:, s], in0=t["u1"][:, s], scalar1=-1.0,
                scalar2=1.0, op0=OP.mult, op1=OP.add,
            )
            nc.vector.tensor_scalar(
                out=t["t2"][:, s], in0=t["u1"][:, s], scalar1=1.0,
                scalar2=None, op0=OP.add,
            )
            mm_eng = nc.gpsimd if eng == 'g' else nc.vector
            mm_eng.tensor_mul(t["mm"][:, s], t["t1"][:, s], t["t2"][:, s])
            nc.scalar.activation(t["sr"][:, s], t["mm"][:, s],
                                 AF.Abs_reciprocal_sqrt)

        def chain_ss(k, ws=None):
            w0, w1 = ws if ws else (0, WC)
            s = slice(w0, w1)
            nc.vector.tensor_mul(ssb[:, k, s], chn[k]["t1"][:, s],
                                 chn[k]["sr"][:, s])

        # ---------------- emission schedule ----------------
        HW_ = WC // 2
        products(0)
        absred(0)
        products(1)
        absred(1)
        ps0 = pe_unit(0)
        tail(0, ps0)
        products(2)
        absred(2)
        ps1 = pe_unit(1)
        tail(1, ps1)
        products(3)
        absred(3)
        ps2 = pe_unit(2)
        tail(2, ps2)
        products(4)
        absred(4)
        ps3 = pe_unit(3)
        tail(3, ps3)
        products(5, (0, HW_))
        ps4 = pe_unit(4)
        tail(4, ps4)
        chain(0, eng='g')
        products(5, (HW_, WC))
        absred(5)
        ps5a = pe_unit(5, (0, HW_))
        tail(5, ps5a, (0, HW_))
        chain(1, (0, HW_), eng='v')
        ps5b = pe_unit(5, (HW_, WC))
        tail(5, ps5b, (HW_, WC))
        chain_ss(0)
        chain(1, (HW_, WC), eng='v')
        chain_ss(1, (0, HW_))
        chain_ss(1, (HW_, WC))

        at = outp.tile([P, NCH, WC], F16)
        nc.scalar.activation(
            out=at, in_=ssb, func=AF.Arctan, accum_out=res[:, 7:8]
        )

        nc.sync.dma_start(out=res_out, in_=res)


def _build():
    nc = bacc.Bacc(
        "TRN2", target_bir_lowering=False, debug=False, num_devices=NCORES
    )
    inputs = nc.dram_tensor(
        "inputs", [C, NCH, HC, 2, B, WC], F16, kind="ExternalInput"
    ).ap()
    identf16 = nc.dram_tensor("identf16", [P, P], F16, kind="ExternalInput").ap()
    res_out = nc.dram_tensor("partials", [P, 8], F32, kind="ExternalOutput").ap()
    with tile.TileContext(nc) as tc:
        _body(tc, inputs, identf16, res_out)
    nc.compile()
    return nc


def _get_nc():
    if "nc" not in _CACHE:
        _CACHE["nc"] = _build()
    return _CACHE["nc"]


def _make_in_maps(predictions, targets):
    p = np.asarray(predictions)
    t = np.asarray(targets)
    ident = np.eye(P, dtype=np.float16)
    in_maps = []
    for i in range(NCORES):
        h0 = i * HC
        # [2, B, C, HC, W] -> [C, NCH, HC, 2, B, WC] fp16: one unit (c, k)
        # is a contiguous 1MB block, 8KB per partition-row covering p and t
        both = np.stack([p[:, :, h0 : h0 + HC, :], t[:, :, h0 : h0 + HC, :]])
        arr = np.ascontiguousarray(
            both.reshape(2, B, C, HC, NCH, WC)
            .transpose(2, 4, 3, 0, 1, 5)
            .astype(np.float16)
        )
        in_maps.append({"inputs": arr, "identf16": ident})
    return in_maps


def _combine(results):
    rsum = 0.0
    atsum = 0.0
    for r in results:
        part = np.asarray(r["partials"], dtype=np.float64)
        rsum += part[:, 0:6].sum()
        atsum += part[:, 7].sum()
    loss = rsum / float(B * C * H * W) + atsum
    return np.asarray(np.float32(loss))


def kernel(predictions, targets, _trace=False):
    nc = _get_nc()
    in_maps = _make_in_maps(predictions, targets)
    if _trace:
        out = run_bass_kernel_spmd(
            nc, in_maps, core_ids=list(range(NCORES)), trace=True
        )
        return _combine(out.results), out
    out = run_bass_kernel_spmd(nc, in_maps, core_ids=list(range(NCORES)))
    return _combine(out.results)
